# revision 28
# baseline (speedup 1.0000x reference)
"""Multi-head attention forward (B=2, S=2048, D=1024, H=16, Hd=64) on 8
Trainium2 NeuronCores.

Sharding: core c handles batch b = c // 4 and the 4 heads (c % 4)*4 .. +4.
Each core computes its heads' Q/K/V projections, attention, and a partial
row-parallel o_proj; the host sums the 4 partial outputs per batch and adds
the output bias.

Key layout decisions (per core, S=2048, Dloc=256 = 4 heads):
  - The host pre-transposes activations to x.T [D, S] so every matmul
    contraction over D streams straight from DRAM; no on-chip transposes.
  - Activations/weights are fed as float32r (full-rate fp32 matmul mode,
    ~1.5e-4 matmul rel-err; DMA of f32r-declared DRAM tensors satisfies the
    rounded-producer rule).
  - qhT/khT are head-pair-stacked: [128 (2 heads x 64), S].
  - Scores are computed transposed (scoresT[sk, q]) so the softmax key-sum
    is the matmul contraction dim; sum-of-exp comes free from a ones
    column appended to V (vh_aug, M=65).
  - Softmax skips the max subtraction: scores ~ N(0,1) at this problem's
    scale, exp stays far from fp32 overflow.
  - The whole attention phase is one software-pipelined stream across all
    (q-block, head-pair) units: scores/exp run LAG iterations ahead of the
    ctx accumulation, and normalize/o_proj chains are emitted inside the
    following unit's scores stream so the PE never idles at boundaries.
"""

import numpy as np

S = 2048
D = 1024
H = 16
HD = 64
B = 2

P = 128
SBLK = 512          # s-positions per block
NSB = S // SBLK     # 4
DCH = D // P        # 8
NKT = S // P        # 16 key tiles
NH = 4              # heads per core
NG = 2              # head-pair groups per core
DLOC = NH * HD      # 256

_program_cache = {}


def _split_excess_waits(nc, mybir, max_waits=1):
    """This walrus build rejects instructions with >1 semaphore wait. Move
    excess waits onto preceding NoOps on the same engine queue (engines are
    strict FIFO, so blocking a NoOp blocks the instruction)."""
    n = 0
    for f in nc.m.functions:
        for bb in f.blocks:
            new = []
            changed = False
            for inst in bb.instructions:
                si = inst.sync_info
                waits = list(si.on_wait) if si is not None else []
                if len(waits) > max_waits:
                    extra = waits[:-max_waits]
                    keep = waits[-max_waits:]
                    for i in range(0, len(extra), max_waits):
                        nop = mybir.InstNoOp(
                            name=f"__waitsplit_{n}", ins=[], outs=[]
                        )
                        n += 1
                        nop.engine = inst.engine
                        nop.sync_info = mybir.SyncInfo(
                            on_wait=extra[i : i + max_waits], on_update=[]
                        )
                        new.append(nop)
                    inst.sync_info = mybir.SyncInfo(
                        on_wait=keep, on_update=list(si.on_update)
                    )
                    changed = True
                new.append(inst)
            if changed:
                bb.instructions = new
    return n


def _build_program():
    import concourse.bass as bass
    import concourse.mybir as mybir
    from concourse.bass import ds, ts
    from concourse.tile import TileContext

    f32 = mybir.dt.float32
    f32r = mybir.dt.float32r
    AF = mybir.ActivationFunctionType

    nc = bass.Bass()
    xqT = nc.declare_dram_parameter("xqT", [D, S], f32r, isOutput=False)
    xkT = nc.declare_dram_parameter("xkT", [D, S], f32r, isOutput=False)
    xvT = nc.declare_dram_parameter("xvT", [D, S], f32r, isOutput=False)
    wq = nc.declare_dram_parameter("wq", [D, DLOC], f32r, isOutput=False)
    wk = nc.declare_dram_parameter("wk", [D, DLOC], f32r, isOutput=False)
    wv = nc.declare_dram_parameter("wv", [D, DLOC], f32r, isOutput=False)
    wo = nc.declare_dram_parameter("wo", [DLOC, D], f32r, isOutput=False)
    bq = nc.declare_dram_parameter("bq", [DLOC], f32, isOutput=False)
    bk = nc.declare_dram_parameter("bk", [DLOC], f32, isOutput=False)
    bv = nc.declare_dram_parameter("bv", [DLOC], f32, isOutput=False)
    y = nc.declare_dram_parameter("y", [S, D], f32, isOutput=True)

    with TileContext(nc) as tc:
        with (
            tc.tile_pool(name="const", bufs=1) as const,
            tc.tile_pool(name="kv", bufs=1) as kv,
            tc.tile_pool(name="xstr", bufs=3) as xstr,
            tc.tile_pool(name="epool", bufs=6) as epool,
            tc.tile_pool(name="cpool", bufs=2) as cpool,
            tc.tile_pool(name="upool", bufs=2) as upool,
            tc.tile_pool(name="rpool", bufs=2) as rpool,
            tc.tile_pool(name="opool", bufs=2) as opool,
            tc.tile_pool(name="ps_k", bufs=2, space="PSUM") as ps_k,
            tc.tile_pool(name="ps_s", bufs=2, space="PSUM") as ps_s,
            tc.tile_pool(name="ps_c", bufs=2, space="PSUM") as ps_c,
        ):
            # ---- warmup: trip the PE HAM activity monitor to full clock
            # while the first DMAs stream in.
            warm_in = const.tile([P, P], f32)
            nc.gpsimd.memset(warm_in, 0.0)
            warm_ps = ps_k.tile([P, SBLK], f32, tag="k", name="warm")
            for _ in range(48):
                nc.tensor.matmul(
                    warm_ps[:, 0:P], warm_in[:], warm_in[:],
                    start=True, stop=True,
                )

            # ---- constants / weights (DMA'd straight in as f32r) ---------
            ones_f32 = const.tile([1, 64], f32)
            nc.gpsimd.memset(ones_f32, 1.0)
            ones_col = const.tile([1, 64], f32r)
            nc.vector.tensor_copy(ones_col[:], ones_f32[:])
            one_sb = const.tile([P, 1], f32)
            nc.gpsimd.memset(one_sb, 1.0)

            def load_weight(dram, name):
                r = const.tile([P, DCH, DLOC], f32r, name=f"w_{name}")
                nc.sync.dma_start(
                    r[:], dram.rearrange("(dc p) n -> p dc n", p=P)
                )
                return r

            # K/V path loads first: they gate the first scores
            wk_sb = load_weight(wk, "k")
            wv_sb = load_weight(wv, "v")
            bk_sb = const.tile([P, NG], f32)
            nc.sync.dma_start(bk_sb[:], bk.rearrange("(g p) -> p g", p=P))
            bv_sb = const.tile([P, NG], f32)
            nc.sync.dma_start(bv_sb[:], bv.rearrange("(g p) -> p g", p=P))
            wq_sb = load_weight(wq, "q")
            bq_sb = const.tile([P, NG], f32)
            nc.sync.dma_start(bq_sb[:], bq.rearrange("(g p) -> p g", p=P))
            # wo is not needed until the first o_proj (~half way in); its
            # DMA is emitted inside the attention stream
            wo_sb = const.tile([P, NG, D], f32r)

            # persistent Q/K/V state
            qhT = kv.tile([P, NG, S], f32r)
            khT = [
                kv.tile([P, S], f32r, tag=f"khT{g}", name=f"khT{g}")
                for g in range(NG)
            ]
            # vh_aug: [sk-part, kt, head*65] with col 64 of each head == 1.0
            vh_aug = kv.tile([P, NKT, NH * 65], f32r)
            vh4 = vh_aug[:].rearrange("p k (h e) -> p k h e", e=65)
            nc.vector.tensor_copy(
                vh4[:, :, :, 64], one_sb[:].to_broadcast([P, NKT, NH])
            )

            def stream_xT(dram, blk, name):
                t = xstr.tile([P, DCH, SBLK], f32r, tag="xstr", name=name)
                view = dram.rearrange("(dc p) s -> p dc s", p=P)
                for dc in range(DCH):
                    nc.sync.dma_start(
                        t[:, dc], view[:, dc, ds(blk * SBLK, SBLK)]
                    )
                return t

            def emit_qproj(qb):
                xq_blk = stream_xT(xqT, qb, "xq")
                for g in range(NG):
                    pq = ps_k.tile([P, SBLK], f32, tag="k", name="pq")
                    for dc in range(DCH):
                        nc.tensor.matmul(
                            pq[:],
                            wq_sb[:, dc, ts(g, P)],
                            xq_blk[:, dc, :],
                            start=(dc == 0),
                            stop=(dc == DCH - 1),
                        )
                    nc.vector.tensor_scalar_add(
                        qhT[:, g, ts(qb, SBLK)], pq[:], bq_sb[:, g : g + 1]
                    )

            def emit_kv(sb):
                xk_blk = stream_xT(xkT, sb, "xk")
                for g in range(NG):
                    pk = ps_k.tile([P, SBLK], f32, tag="k", name="pk")
                    for dc in range(DCH):
                        nc.tensor.matmul(
                            pk[:],
                            wk_sb[:, dc, ts(g, P)],
                            xk_blk[:, dc, :],
                            start=(dc == 0),
                            stop=(dc == DCH - 1),
                        )
                    nc.vector.tensor_scalar_add(
                        khT[g][:, ts(sb, SBLK)], pk[:], bk_sb[:, g : g + 1]
                    )
                xv_blk = stream_xT(xvT, sb, "xv")
                for ss in range(4):
                    pv = ps_k.tile([P, DLOC], f32, tag="k", name="pv")
                    for dc in range(DCH):
                        nc.tensor.matmul(
                            pv[:],
                            xv_blk[:, dc, ts(ss, P)],
                            wv_sb[:, dc, :],
                            start=(dc == 0),
                            stop=(dc == DCH - 1),
                        )
                    kt = sb * 4 + ss
                    nc.vector.tensor_copy(
                        vh4[:, kt, :, 0:64],
                        pv[:].rearrange("p (h e) -> p h e", e=64),
                    )

            # ---- attention + o_proj: one continuous pipeline -------------
            LAG = 4
            pcs = {}
            exs = {}
            ctx2s = {}
            ctxus = {}
            rcs = {}

            def emit_scores_exp(qb, g, kt):
                ps2 = ps_s.tile([P, 2 * SBLK], f32, tag="s", name="ps2")
                for hh in range(2):
                    hr = hh * 64
                    nc.tensor.matmul(
                        ps2[:, ts(hh, SBLK)],
                        khT[g][hr : hr + 64, ts(kt, P)],
                        qhT[hr : hr + 64, g, ts(qb, SBLK)],
                        start=True,
                        stop=True,
                        tile_position=(hr, 0),
                    )
                ex = epool.tile([P, 2 * SBLK], f32r, name="ex")
                nc.scalar.activation(ex[:], ps2[:], AF.Exp, scale=0.125)
                exs[(qb, g, kt)] = ex

            def emit_ctx(qb, g, kt):
                if kt == 0:
                    pcs[(qb, g)] = [
                        ps_c.tile([P, SBLK], f32, tag="c", name=f"pc{hh}")
                        for hh in range(2)
                    ]
                    if g == 0:
                        ctx2s[qb] = cpool.tile(
                            [P, NG, SBLK], f32r, name="ctx2"
                        )
                ex = exs.pop((qb, g, kt))
                for hh in range(2):
                    h = 2 * g + hh
                    nc.tensor.matmul(
                        pcs[(qb, g)][hh][0:65, :],
                        vh_aug[:, kt, h * 65 : h * 65 + 65],
                        ex[:, ts(hh, SBLK)],
                        start=(kt == 0),
                        stop=(kt == NKT - 1),
                    )

            def emit_evac(qb, g):
                # Evacuate the finished ctx psum pair to SBUF with two
                # cheap DVE copies: frees the accumulation banks without
                # waiting on the normalize. 1/Z runs on the DVE (exact
                # iterative divide, ~3.4us for [1,512]) — its latency hides
                # behind the following unit's stream, and it keeps the ACT
                # queue free for the pipeline-critical exps.
                us = []
                for hh in range(2):
                    pc = pcs[(qb, g)][hh]
                    u = upool.tile([65, SBLK], f32, tag=f"u{hh}", name="u")
                    nc.vector.tensor_copy(u[:], pc[0:65, :])
                    us.append(u)
                del pcs[(qb, g)]
                ctxus[(qb, g)] = us
                for hh in range(2):
                    if (qb, g) == (NSB - 1, NG - 1):
                        # tail unit: ACT is idle by now; exp(-ln Z) avoids
                        # the 3.4us DVE reciprocal on the critical tail
                        lz = rpool.tile([1, SBLK], f32, name="lz")
                        nc.scalar.activation(lz[:], us[hh][64:65, :], AF.Ln)
                        rc = rpool.tile([1, SBLK], f32, name="rc")
                        nc.scalar.activation(
                            rc[:], lz[:], AF.Exp, scale=-1.0
                        )
                    else:
                        rc = rpool.tile([1, SBLK], f32, name="rc")
                        nc.vector.reciprocal(rc[:], us[hh][64:65, :])
                    rcs[(qb, g, hh)] = rc

            def emit_norm_rest(qb, g):
                ctx2 = ctx2s[qb]
                us = ctxus.pop((qb, g))
                for hh in range(2):
                    hr = hh * 64
                    rc = rcs.pop((qb, g, hh))
                    pb = ps_s.tile([64, SBLK], f32, tag="s", name="pb")
                    nc.tensor.matmul(
                        pb[:], ones_f32[:], rc[:], start=True, stop=True
                    )
                    rb = rpool.tile([64, SBLK], f32, name="rb")
                    nc.vector.tensor_copy(rb[:], pb[:])
                    nc.vector.tensor_mul(
                        ctx2[hr : hr + 64, g, :], us[hh][0:64, :], rb[:]
                    )
                nc.vector.tensor_scalar_add(
                    ctx2[:, g, :], ctx2[:, g, :], bv_sb[:, g : g + 1]
                )

            def emit_o_proj(qb):
                ctx2 = ctx2s.pop(qb)
                for qs in range(4):
                    ost = opool.tile([P, D], f32, name="ost")
                    for nch in range(2):
                        po = ps_k.tile([P, SBLK], f32, tag="k", name="po")
                        for g in range(NG):
                            nc.tensor.matmul(
                                po[:],
                                ctx2[:, g, ts(qs, P)],
                                wo_sb[:, g, ts(nch, SBLK)],
                                start=(g == 0),
                                stop=(g == NG - 1),
                            )
                        nc.vector.tensor_copy(ost[:, ts(nch, SBLK)], po[:])
                    nc.sync.dma_start(
                        y[ds(qb * SBLK + qs * P, P), :], ost[:]
                    )

            # K/V blocks and later q-block projections are emitted INSIDE
            # the attention stream: the first unit's scores chase the K/V
            # production block by block, so ACT starts exp-ing ~40us
            # earlier, and the projections act as PE filler between
            # ACT-paced iterations.
            emit_kv(0)
            emit_qproj(0)
            steps = [
                (qb, g, kt)
                for qb in range(NSB)
                for g in range(NG)
                for kt in range(NKT)
            ]
            for i, (qb, g, kt) in enumerate(steps):
                emit_scores_exp(qb, g, kt)
                if i == 2:
                    emit_kv(1)
                elif i == 4:
                    nc.sync.dma_start(
                        wo_sb[:], wo.rearrange("(g p) n -> p g n", p=P)
                    )
                elif i == 6:
                    emit_kv(2)
                elif i == 10:
                    emit_kv(3)
                elif i == 18:
                    emit_qproj(1)
                elif i == 34:
                    emit_qproj(2)
                elif i == 66:
                    emit_qproj(3)
                if i >= LAG:
                    pqb, pg, pkt = steps[i - LAG]
                    emit_ctx(pqb, pg, pkt)
                    if pkt == NKT - 1:
                        emit_evac(pqb, pg)
                if kt == 8:
                    # the previous unit's reciprocal has cleared DVE by now
                    if g == 1:
                        emit_norm_rest(qb, 0)
                    elif qb > 0:
                        emit_norm_rest(qb - 1, 1)
                if kt == 12 and g == 0 and qb > 0:
                    emit_o_proj(qb - 1)
            for j in range(len(steps) - LAG, len(steps)):
                qb, g, kt = steps[j]
                emit_ctx(qb, g, kt)
                if kt == NKT - 1:
                    emit_evac(qb, g)

            emit_norm_rest(NSB - 1, 1)
            emit_o_proj(NSB - 1)

    import concourse.mybir as mybir
    from concourse.bass import ds, ts
    from concourse.tile import TileContext

    f32 = mybir.dt.float32
    f32r = mybir.dt.float32r
    AF = mybir.ActivationFunctionType

    nc = bass.Bass()
    xqT = nc.declare_dram_parameter("xqT", [D, S], f32r, isOutput=False)
    xkT = nc.declare_dram_parameter("xkT", [D, S], f32r, isOutput=False)
    xvT = nc.declare_dram_parameter("xvT", [D, S], f32r, isOutput=False)
    wq = nc.declare_dram_parameter("wq", [D, DLOC], f32r, isOutput=False)
    wk = nc.declare_dram_parameter("wk", [D, DLOC], f32r, isOutput=False)
    wv = nc.declare_dram_parameter("wv", [D, DLOC], f32r, isOutput=False)
    wo = nc.declare_dram_parameter("wo", [DLOC, D], f32r, isOutput=False)
    bq = nc.declare_dram_parameter("bq", [DLOC], f32, isOutput=False)
    bk = nc.declare_dram_parameter("bk", [DLOC], f32, isOutput=False)
    bv = nc.declare_dram_parameter("bv", [DLOC], f32, isOutput=False)
    y = nc.declare_dram_parameter("y", [S, D], f32, isOutput=True)

    with TileContext(nc) as tc:
        with (
            tc.tile_pool(name="const", bufs=1) as const,
            tc.tile_pool(name="kv", bufs=1) as kv,
            tc.tile_pool(name="xstr", bufs=3) as xstr,
            tc.tile_pool(name="epool", bufs=6) as epool,
            tc.tile_pool(name="cpool", bufs=2) as cpool,
            tc.tile_pool(name="upool", bufs=2) as upool,
            tc.tile_pool(name="rpool", bufs=2) as rpool,
            tc.tile_pool(name="opool", bufs=2) as opool,
            tc.tile_pool(name="ps_k", bufs=2, space="PSUM") as ps_k,
            tc.tile_pool(name="ps_s", bufs=2, space="PSUM") as ps_s,
            tc.tile_pool(name="ps_c", bufs=2, space="PSUM") as ps_c,
        ):
            # ---- warmup: trip the PE HAM activity monitor to full clock
            # while the first DMAs stream in.
            warm_in = const.tile([P, P], f32)
            nc.gpsimd.memset(warm_in, 0.0)
            warm_ps = ps_k.tile([P, SBLK], f32, tag="k", name="warm")
            for _ in range(48):
                nc.tensor.matmul(
                    warm_ps[:, 0:P], warm_in[:], warm_in[:],
                    start=True, stop=True,
                )

            # ---- constants / weights (DMA'd straight in as f32r) ---------
            ones_f32 = const.tile([1, 64], f32)
            nc.gpsimd.memset(ones_f32, 1.0)
            ones_col = const.tile([1, 64], f32r)
            nc.vector.tensor_copy(ones_col[:], ones_f32[:])
            one_sb = const.tile([P, 1], f32)
            nc.gpsimd.memset(one_sb, 1.0)

            def load_weight(dram, name):
                r = const.tile([P, DCH, DLOC], f32r, name=f"w_{name}")
                nc.sync.dma_start(
                    r[:], dram.rearrange("(dc p) n -> p dc n", p=P)
                )
                return r

            # K/V path loads first: they gate the first scores
            wk_sb = load_weight(wk, "k")
            wv_sb = load_weight(wv, "v")
            bk_sb = const.tile([P, NG], f32)
            nc.sync.dma_start(bk_sb[:], bk.rearrange("(g p) -> p g", p=P))
            bv_sb = const.tile([P, NG], f32)
            nc.sync.dma_start(bv_sb[:], bv.rearrange("(g p) -> p g", p=P))
            wq_sb = load_weight(wq, "q")
            bq_sb = const.tile([P, NG], f32)
            nc.sync.dma_start(bq_sb[:], bq.rearrange("(g p) -> p g", p=P))
            # wo is not needed until the first o_proj (~half way in); its
            # DMA is emitted inside the attention stream
            wo_sb = const.tile([P, NG, D], f32r)

            # persistent Q/K/V state
            qhT = kv.tile([P, NG, S], f32r)
            khT = [
                kv.tile([P, S], f32r, tag=f"khT{g}", name=f"khT{g}")
                for g in range(NG)
            ]
            # vh_aug: [sk-part, kt, head*65] with col 64 of each head == 1.0
            vh_aug = kv.tile([P, NKT, NH * 65], f32r)
            vh4 = vh_aug[:].rearrange("p k (h e) -> p k h e", e=65)
            nc.vector.tensor_copy(
                vh4[:, :, :, 64], one_sb[:].to_broadcast([P, NKT, NH])
            )

            def stream_xT(dram, blk, name):
                t = xstr.tile([P, DCH, SBLK], f32r, tag="xstr", name=name)
                view = dram.rearrange("(dc p) s -> p dc s", p=P)
                for dc in range(DCH):
                    nc.sync.dma_start(
                        t[:, dc], view[:, dc, ds(blk * SBLK, SBLK)]
                    )
                return t

            def emit_qproj(qb):
                xq_blk = stream_xT(xqT, qb, "xq")
                for g in range(NG):
                    pq = ps_k.tile([P, SBLK], f32, tag="k", name="pq")
                    for dc in range(DCH):
                        nc.tensor.matmul(
                            pq[:],
                            wq_sb[:, dc, ts(g, P)],
                            xq_blk[:, dc, :],
                            start=(dc == 0),
                            stop=(dc == DCH - 1),
                        )
                    nc.vector.tensor_scalar_add(
                        qhT[:, g, ts(qb, SBLK)], pq[:], bq_sb[:, g : g + 1]
                    )

            def emit_kv(sb):
                xk_blk = stream_xT(xkT, sb, "xk")
                for g in range(NG):
                    pk = ps_k.tile([P, SBLK], f32, tag="k", name="pk")
                    for dc in range(DCH):
                        nc.tensor.matmul(
                            pk[:],
                            wk_sb[:, dc, ts(g, P)],
                            xk_blk[:, dc, :],
                            start=(dc == 0),
                            stop=(dc == DCH - 1),
                        )
                    nc.vector.tensor_scalar_add(
                        khT[g][:, ts(sb, SBLK)], pk[:], bk_sb[:, g : g + 1]
                    )
                xv_blk = stream_xT(xvT, sb, "xv")
                for ss in range(4):
                    pv = ps_k.tile([P, DLOC], f32, tag="k", name="pv")
                    for dc in range(DCH):
                        nc.tensor.matmul(
                            pv[:],
                            xv_blk[:, dc, ts(ss, P)],
                            wv_sb[:, dc, :],
                            start=(dc == 0),
                            stop=(dc == DCH - 1),
                        )
                    kt = sb * 4 + ss
                    nc.vector.tensor_copy(
                        vh4[:, kt, :, 0:64],
                        pv[:].rearrange("p (h e) -> p h e", e=64),
                    )

            # ---- attention + o_proj: one continuous pipeline -------------
            LAG = 4
            pcs = {}
            exs = {}
            ctx2s = {}
            ctxus = {}
            rcs = {}

            def emit_scores_exp(qb, g, kt):
                ps2 = ps_s.tile([P, 2 * SBLK], f32, tag="s", name="ps2")
                for hh in range(2):
                    hr = hh * 64
                    nc.tensor.matmul(
                        ps2[:, ts(hh, SBLK)],
                        khT[g][hr : hr + 64, ts(kt, P)],
                        qhT[hr : hr + 64, g, ts(qb, SBLK)],
                        start=True,
                        stop=True,
                        tile_position=(hr, 0),
                    )
                ex = epool.tile([P, 2 * SBLK], f32r, name="ex")
                nc.scalar.activation(ex[:], ps2[:], AF.Exp, scale=0.125)
                exs[(qb, g, kt)] = ex

            def emit_ctx(qb, g, kt):
                if kt == 0:
                    pcs[(qb, g)] = [
                        ps_c.tile([P, SBLK], f32, tag="c", name=f"pc{hh}")
                        for hh in range(2)
                    ]
                    if g == 0:
                        ctx2s[qb] = cpool.tile(
                            [P, NG, SBLK], f32r, name="ctx2"
                        )
                ex = exs.pop((qb, g, kt))
                for hh in range(2):
                    h = 2 * g + hh
                    nc.tensor.matmul(
                        pcs[(qb, g)][hh][0:65, :],
                        vh_aug[:, kt, h * 65 : h * 65 + 65],
                        ex[:, ts(hh, SBLK)],
                        start=(kt == 0),
                        stop=(kt == NKT - 1),
                    )

            def emit_evac(qb, g):
                # Evacuate the finished ctx psum pair to SBUF with two
                # cheap DVE copies: frees the accumulation banks without
                # waiting on the normalize. 1/Z runs on the DVE (exact
                # iterative divide, ~3.4us for [1,512]) — its latency hides
                # behind the following unit's stream, and it keeps the ACT
                # queue free for the pipeline-critical exps.
                us = []
                for hh in range(2):
                    pc = pcs[(qb, g)][hh]
                    u = upool.tile([65, SBLK], f32, tag=f"u{hh}", name="u")
                    nc.vector.tensor_copy(u[:], pc[0:65, :])
                    us.append(u)
                del pcs[(qb, g)]
                ctxus[(qb, g)] = us
                for hh in range(2):
                    if (qb, g) == (NSB - 1, NG - 1):
                        # tail unit: ACT is idle by now; exp(-ln Z) avoids
                        # the 3.4us DVE reciprocal on the critical tail
                        lz = rpool.tile([1, SBLK], f32, name="lz")
                        nc.scalar.activation(lz[:], us[hh][64:65, :], AF.Ln)
                        rc = rpool.tile([1, SBLK], f32, name="rc")
                        nc.scalar.activation(
                            rc[:], lz[:], AF.Exp, scale=-1.0
                        )
                    else:
                        rc = rpool.tile([1, SBLK], f32, name="rc")
                        nc.vector.reciprocal(rc[:], us[hh][64:65, :])
                    rcs[(qb, g, hh)] = rc

            def emit_norm_rest(qb, g):
                ctx2 = ctx2s[qb]
                us = ctxus.pop((qb, g))
                for hh in range(2):
                    hr = hh * 64
                    rc = rcs.pop((qb, g, hh))
                    pb = ps_s.tile([64, SBLK], f32, tag="s", name="pb")
                    nc.tensor.matmul(
                        pb[:], ones_f32[:], rc[:], start=True, stop=True
                    )
                    rb = rpool.tile([64, SBLK], f32, name="rb")
                    nc.vector.tensor_copy(rb[:], pb[:])
                    nc.vector.tensor_mul(
                        ctx2[hr : hr + 64, g, :], us[hh][0:64, :], rb[:]
                    )
                nc.vector.tensor_scalar_add(
                    ctx2[:, g, :], ctx2[:, g, :], bv_sb[:, g : g + 1]
                )

            def emit_o_proj(qb):
                ctx2 = ctx2s.pop(qb)
                for qs in range(4):
                    ost = opool.tile([P, D], f32, name="ost")
                    for nch in range(2):
                        po = ps_k.tile([P, SBLK], f32, tag="k", name="po")
                        for g in range(NG):
                            nc.tensor.matmul(
                                po[:],
                                ctx2[:, g, ts(qs, P)],
                                wo_sb[:, g, ts(nch, SBLK)],
                                start=(g == 0),
                                stop=(g == NG - 1),
                            )
                        nc.vector.tensor_copy(ost[:, ts(nch, SBLK)], po[:])
                    nc.sync.dma_start(
                        y[ds(qb * SBLK + qs * P, P), :], ost[:]
                    )

            # K/V blocks and later q-block projections are emitted INSIDE
            # the attention stream: the first unit's scores chase the K/V
            # production block by block, so ACT starts exp-ing ~40us
            # earlier, and the projections act as PE filler between
            # ACT-paced iterations.
            emit_kv(0)
            emit_qproj(0)
            steps = [
                (qb, g, kt)
                for qb in range(NSB)
                for g in range(NG)
                for kt in range(NKT)
            ]
            for i, (qb, g, kt) in enumerate(steps):
                emit_scores_exp(qb, g, kt)
                if i == 2:
                    emit_kv(1)
                elif i == 4:
                    nc.sync.dma_start(
                        wo_sb[:], wo.rearrange("(g p) n -> p g n", p=P)
                    )
                elif i == 6:
                    emit_kv(2)
                elif i == 10:
                    emit_kv(3)
                elif i == 18:
                    emit_qproj(1)
                elif i == 34:
                    emit_qproj(2)
                elif i == 66:
                    emit_qproj(3)
                if i >= LAG:
                    pqb, pg, pkt = steps[i - LAG]
                    emit_ctx(pqb, pg, pkt)
                    if pkt == NKT - 1:
                        emit_evac(pqb, pg)
                if kt == 8:
                    # the previous unit's reciprocal has cleared DVE by now
                    if g == 1:
                        emit_norm_rest(qb, 0)
                    elif qb > 0:
                        emit_norm_rest(qb - 1, 1)
                if kt == 12 and g == 0 and qb > 0:
                    emit_o_proj(qb - 1)
            for j in range(len(steps) - LAG, len(steps)):
                qb, g, kt = steps[j]
                emit_ctx(qb, g, kt)
                if kt == NKT - 1:
                    emit_evac(qb, g)

            # fused tail for the last unit: normalize and o_proj pipeline
            # per 128-column chunk instead of running the whole normalize
            # before the first o_proj matmul
            qb, g = NSB - 1, 1
            ctx2 = ctx2s.pop(qb)
            us = ctxus.pop((qb, g))
            rbs = []
            for hh in range(2):
                rc = rcs.pop((qb, g, hh))
                pb = ps_s.tile([64, SBLK], f32, tag="s", name="pb")
                nc.tensor.matmul(
                    pb[:], ones_f32[:], rc[:], start=True, stop=True
                )
                rb = rpool.tile([64, SBLK], f32, name="rb")
                nc.vector.tensor_copy(rb[:], pb[:])
                rbs.append(rb)
            for qs in range(4):
                qsl = ts(qs, P)
                for hh in range(2):
                    hr = hh * 64
                    nc.vector.tensor_mul(
                        ctx2[hr : hr + 64, g, qsl],
                        us[hh][0:64, qsl],
                        rbs[hh][:, qsl],
                    )
                nc.vector.tensor_scalar_add(
                    ctx2[:, g, qsl], ctx2[:, g, qsl], bv_sb[:, g : g + 1]
                )
                ost = opool.tile([P, D], f32, name="ost")
                for nch in range(2):
                    po = ps_k.tile([P, SBLK], f32, tag="k", name="po")
                    for gg in range(NG):
                        nc.tensor.matmul(
                            po[:],
                            ctx2[:, gg, qsl],
                            wo_sb[:, gg, ts(nch, SBLK)],
                            start=(gg == 0),
                            stop=(gg == NG - 1),
                        )
                    nc.vector.tensor_copy(ost[:, ts(nch, SBLK)], po[:])
                nc.sync.dma_start(y[ds(qb * SBLK + qs * P, P), :], ost[:])

    import concourse.mybir as mybir

    _split_excess_waits(nc, mybir)
    return nc


def kernel(q, k, v, Wq, bq, Wk, bk, Wv, bv, Wo, bo):
    from concourse.bass_utils import run_bass_kernel_spmd

    q = np.asarray(q, dtype=np.float32)
    k = np.asarray(k, dtype=np.float32)
    v = np.asarray(v, dtype=np.float32)
    Wq = np.asarray(Wq, dtype=np.float32)
    Wk = np.asarray(Wk, dtype=np.float32)
    Wv = np.asarray(Wv, dtype=np.float32)
    Wo = np.asarray(Wo, dtype=np.float32)
    bq = np.asarray(bq, dtype=np.float32)
    bk = np.asarray(bk, dtype=np.float32)
    bv = np.asarray(bv, dtype=np.float32)
    bo = np.asarray(bo, dtype=np.float32)

    if "nc" not in _program_cache:
        _program_cache["nc"] = _build_program()
    nc = _program_cache["nc"]

    qT = [np.ascontiguousarray(q[b].T) for b in range(B)]
    kT = [np.ascontiguousarray(k[b].T) for b in range(B)]
    vT = [np.ascontiguousarray(v[b].T) for b in range(B)]

    in_maps = []
    for c in range(8):
        b, hg = c // 4, c % 4
        cols = slice(DLOC * hg, DLOC * (hg + 1))
        in_maps.append(
            {
                "xqT": qT[b],
                "xkT": kT[b],
                "xvT": vT[b],
                "wq": np.ascontiguousarray(Wq[:, cols]),
                "wk": np.ascontiguousarray(Wk[:, cols]),
                "wv": np.ascontiguousarray(Wv[:, cols]),
                "wo": np.ascontiguousarray(Wo[cols, :]),
                "bq": np.ascontiguousarray(bq[cols]),
                "bk": np.ascontiguousarray(bk[cols]),
                "bv": np.ascontiguousarray(bv[cols]),
            }
        )

    global _last_in_maps
    _last_in_maps = in_maps

    res = run_bass_kernel_spmd(nc, in_maps, list(range(8)))

    out = np.empty((B, S, D), np.float32)
    for b in range(B):
        acc = res.results[4 * b]["y"].astype(np.float32).copy()
        for hg in range(1, 4):
            acc += res.results[4 * b + hg]["y"]
        out[b] = acc + bo[None, :]
    return out


# revision 29
# speedup vs baseline: 1.0188x; 1.0188x over previous
"""Multi-head attention forward (B=2, S=2048, D=1024, H=16, Hd=64) on 8
Trainium2 NeuronCores.

Sharding: core c handles batch b = c // 4 and the 4 heads (c % 4)*4 .. +4.
Each core computes its heads' Q/K/V projections, attention, and a partial
row-parallel o_proj; the host sums the 4 partial outputs per batch and adds
the output bias.

Key layout decisions (per core, S=2048, Dloc=256 = 4 heads):
  - The host pre-transposes activations to x.T [D, S] so every matmul
    contraction over D streams straight from DRAM; no on-chip transposes.
  - Activations/weights are fed as float32r (full-rate fp32 matmul mode,
    ~1.5e-4 matmul rel-err; DMA of f32r-declared DRAM tensors satisfies the
    rounded-producer rule).
  - qhT/khT are head-pair-stacked: [128 (2 heads x 64), S].
  - Scores are computed transposed (scoresT[sk, q]) so the softmax key-sum
    is the matmul contraction dim; sum-of-exp comes free from a ones
    column appended to V (vh_aug, M=65).
  - Softmax skips the max subtraction: scores ~ N(0,1) at this problem's
    scale, exp stays far from fp32 overflow.
  - The whole attention phase is one software-pipelined stream across all
    (q-block, head-pair) units: scores/exp run LAG iterations ahead of the
    ctx accumulation, and normalize/o_proj chains are emitted inside the
    following unit's scores stream so the PE never idles at boundaries.
"""

import numpy as np

S = 2048
D = 1024
H = 16
HD = 64
B = 2

P = 128
SBLK = 512          # s-positions per block
NSB = S // SBLK     # 4
DCH = D // P        # 8
NKT = S // P        # 16 key tiles
NH = 4              # heads per core
NG = 2              # head-pair groups per core
DLOC = NH * HD      # 256

_program_cache = {}


def _split_excess_waits(nc, mybir, max_waits=1):
    """This walrus build rejects instructions with >1 semaphore wait. Move
    excess waits onto preceding NoOps on the same engine queue (engines are
    strict FIFO, so blocking a NoOp blocks the instruction)."""
    n = 0
    for f in nc.m.functions:
        for bb in f.blocks:
            new = []
            changed = False
            for inst in bb.instructions:
                si = inst.sync_info
                waits = list(si.on_wait) if si is not None else []
                if len(waits) > max_waits:
                    extra = waits[:-max_waits]
                    keep = waits[-max_waits:]
                    for i in range(0, len(extra), max_waits):
                        nop = mybir.InstNoOp(
                            name=f"__waitsplit_{n}", ins=[], outs=[]
                        )
                        n += 1
                        nop.engine = inst.engine
                        nop.sync_info = mybir.SyncInfo(
                            on_wait=extra[i : i + max_waits], on_update=[]
                        )
                        new.append(nop)
                    inst.sync_info = mybir.SyncInfo(
                        on_wait=keep, on_update=list(si.on_update)
                    )
                    changed = True
                new.append(inst)
            if changed:
                bb.instructions = new
    return n


def _build_program():
    import concourse.bass as bass
    import concourse.mybir as mybir
    from concourse.bass import ds, ts
    from concourse.tile import TileContext

    f32 = mybir.dt.float32
    f32r = mybir.dt.float32r
    AF = mybir.ActivationFunctionType

    nc = bass.Bass()
    xqT = nc.declare_dram_parameter("xqT", [D, S], f32r, isOutput=False)
    xkT = nc.declare_dram_parameter("xkT", [D, S], f32r, isOutput=False)
    xvT = nc.declare_dram_parameter("xvT", [D, S], f32r, isOutput=False)
    wq = nc.declare_dram_parameter("wq", [D, DLOC], f32r, isOutput=False)
    wk = nc.declare_dram_parameter("wk", [D, DLOC], f32r, isOutput=False)
    wv = nc.declare_dram_parameter("wv", [D, DLOC], f32r, isOutput=False)
    wo = nc.declare_dram_parameter("wo", [DLOC, D], f32r, isOutput=False)
    bq = nc.declare_dram_parameter("bq", [DLOC], f32, isOutput=False)
    bk = nc.declare_dram_parameter("bk", [DLOC], f32, isOutput=False)
    bv = nc.declare_dram_parameter("bv", [DLOC], f32, isOutput=False)
    y = nc.declare_dram_parameter("y", [S, D], f32, isOutput=True)

    with TileContext(nc) as tc:
        with (
            tc.tile_pool(name="const", bufs=1) as const,
            tc.tile_pool(name="kv", bufs=1) as kv,
            tc.tile_pool(name="xstr", bufs=3) as xstr,
            tc.tile_pool(name="epool", bufs=5) as epool,
            tc.tile_pool(name="cpool", bufs=2) as cpool,
            tc.tile_pool(name="upool", bufs=2) as upool,
            tc.tile_pool(name="rpool", bufs=2) as rpool,
            tc.tile_pool(name="opool", bufs=2) as opool,
            tc.tile_pool(name="ps_k", bufs=2, space="PSUM") as ps_k,
            tc.tile_pool(name="ps_s", bufs=2, space="PSUM") as ps_s,
            tc.tile_pool(name="ps_c", bufs=2, space="PSUM") as ps_c,
        ):
            # ---- warmup: trip the PE HAM activity monitor to full clock
            # while the first DMAs stream in.
            warm_in = const.tile([P, P], f32)
            nc.gpsimd.memset(warm_in, 0.0)
            warm_ps = ps_k.tile([P, SBLK], f32, tag="k", name="warm")
            for _ in range(48):
                nc.tensor.matmul(
                    warm_ps[:, 0:P], warm_in[:], warm_in[:],
                    start=True, stop=True,
                )

            # ---- constants / weights (DMA'd straight in as f32r) ---------
            ones_f32 = const.tile([1, 64], f32)
            nc.gpsimd.memset(ones_f32, 1.0)
            ones_col = const.tile([1, 64], f32r)
            nc.vector.tensor_copy(ones_col[:], ones_f32[:])
            one_sb = const.tile([P, 1], f32)
            nc.gpsimd.memset(one_sb, 1.0)

            def load_weight(dram, name):
                r = const.tile([P, DCH, DLOC], f32r, name=f"w_{name}")
                nc.sync.dma_start(
                    r[:], dram.rearrange("(dc p) n -> p dc n", p=P)
                )
                return r

            # K/V path loads first: they gate the first scores
            wk_sb = load_weight(wk, "k")
            wv_sb = load_weight(wv, "v")
            bk_sb = const.tile([P, NG], f32)
            nc.sync.dma_start(bk_sb[:], bk.rearrange("(g p) -> p g", p=P))
            bv_sb = const.tile([P, NG], f32)
            nc.sync.dma_start(bv_sb[:], bv.rearrange("(g p) -> p g", p=P))
            wq_sb = load_weight(wq, "q")
            bq_sb = const.tile([P, NG], f32)
            nc.sync.dma_start(bq_sb[:], bq.rearrange("(g p) -> p g", p=P))
            # wo is not needed until the first o_proj (~half way in); its
            # DMA is emitted inside the attention stream
            wo_sb = const.tile([P, NG, D], f32r)

            # persistent Q/K/V state
            qhT = kv.tile([P, NG, S], f32r)
            khT = [
                kv.tile([P, S], f32r, tag=f"khT{g}", name=f"khT{g}")
                for g in range(NG)
            ]
            # vh_aug: [sk-part, kt, head*65] with col 64 of each head == 1.0
            vh_aug = kv.tile([P, NKT, NH * 65], f32r)
            vh4 = vh_aug[:].rearrange("p k (h e) -> p k h e", e=65)
            nc.vector.tensor_copy(
                vh4[:, :, :, 64], one_sb[:].to_broadcast([P, NKT, NH])
            )

            def stream_xT(dram, blk, name):
                t = xstr.tile([P, DCH, SBLK], f32r, tag="xstr", name=name)
                view = dram.rearrange("(dc p) s -> p dc s", p=P)
                for dc in range(DCH):
                    nc.sync.dma_start(
                        t[:, dc], view[:, dc, ds(blk * SBLK, SBLK)]
                    )
                return t

            def emit_qproj(qb):
                xq_blk = stream_xT(xqT, qb, "xq")
                for g in range(NG):
                    pq = ps_k.tile([P, SBLK], f32, tag="k", name="pq")
                    for dc in range(DCH):
                        nc.tensor.matmul(
                            pq[:],
                            wq_sb[:, dc, ts(g, P)],
                            xq_blk[:, dc, :],
                            start=(dc == 0),
                            stop=(dc == DCH - 1),
                        )
                    nc.vector.tensor_scalar_add(
                        qhT[:, g, ts(qb, SBLK)], pq[:], bq_sb[:, g : g + 1]
                    )

            def emit_kv(sb):
                xk_blk = stream_xT(xkT, sb, "xk")
                for g in range(NG):
                    pk = ps_k.tile([P, SBLK], f32, tag="k", name="pk")
                    for dc in range(DCH):
                        nc.tensor.matmul(
                            pk[:],
                            wk_sb[:, dc, ts(g, P)],
                            xk_blk[:, dc, :],
                            start=(dc == 0),
                            stop=(dc == DCH - 1),
                        )
                    nc.vector.tensor_scalar_add(
                        khT[g][:, ts(sb, SBLK)], pk[:], bk_sb[:, g : g + 1]
                    )
                xv_blk = stream_xT(xvT, sb, "xv")
                for ss in range(4):
                    pv = ps_k.tile([P, DLOC], f32, tag="k", name="pv")
                    for dc in range(DCH):
                        nc.tensor.matmul(
                            pv[:],
                            xv_blk[:, dc, ts(ss, P)],
                            wv_sb[:, dc, :],
                            start=(dc == 0),
                            stop=(dc == DCH - 1),
                        )
                    kt = sb * 4 + ss
                    nc.vector.tensor_copy(
                        vh4[:, kt, :, 0:64],
                        pv[:].rearrange("p (h e) -> p h e", e=64),
                    )

            # ---- attention + o_proj: one continuous pipeline -------------
            LAG = 3
            pcs = {}
            exs = {}
            ctx2s = {}
            ctxus = {}
            rcs = {}

            def emit_scores_exp(qb, g, kt):
                ps2 = ps_s.tile([P, 2 * SBLK], f32, tag="s", name="ps2")
                for hh in range(2):
                    hr = hh * 64
                    nc.tensor.matmul(
                        ps2[:, ts(hh, SBLK)],
                        khT[g][hr : hr + 64, ts(kt, P)],
                        qhT[hr : hr + 64, g, ts(qb, SBLK)],
                        start=True,
                        stop=True,
                        tile_position=(hr, 0),
                    )
                ex = epool.tile([P, 2 * SBLK], f32r, name="ex")
                nc.scalar.activation(ex[:], ps2[:], AF.Exp, scale=0.125)
                exs[(qb, g, kt)] = ex

            def emit_ctx(qb, g, kt):
                if kt == 0:
                    pcs[(qb, g)] = [
                        ps_c.tile([P, SBLK], f32, tag="c", name=f"pc{hh}")
                        for hh in range(2)
                    ]
                    if g == 0:
                        ctx2s[qb] = cpool.tile(
                            [P, NG, SBLK], f32r, name="ctx2"
                        )
                ex = exs.pop((qb, g, kt))
                for hh in range(2):
                    h = 2 * g + hh
                    nc.tensor.matmul(
                        pcs[(qb, g)][hh][0:65, :],
                        vh_aug[:, kt, h * 65 : h * 65 + 65],
                        ex[:, ts(hh, SBLK)],
                        start=(kt == 0),
                        stop=(kt == NKT - 1),
                    )

            def emit_evac(qb, g):
                # Evacuate the finished ctx psum pair to SBUF with two
                # cheap DVE copies: frees the accumulation banks without
                # waiting on the normalize. 1/Z runs on the DVE (exact
                # iterative divide, ~3.4us for [1,512]) — its latency hides
                # behind the following unit's stream, and it keeps the ACT
                # queue free for the pipeline-critical exps.
                us = []
                for hh in range(2):
                    pc = pcs[(qb, g)][hh]
                    u = upool.tile([65, SBLK], f32, tag=f"u{hh}", name="u")
                    nc.vector.tensor_copy(u[:], pc[0:65, :])
                    us.append(u)
                del pcs[(qb, g)]
                ctxus[(qb, g)] = us
                for hh in range(2):
                    if (qb, g) == (NSB - 1, NG - 1):
                        # tail unit: ACT is idle by now; exp(-ln Z) avoids
                        # the 3.4us DVE reciprocal on the critical tail
                        lz = rpool.tile([1, SBLK], f32, name="lz")
                        nc.scalar.activation(lz[:], us[hh][64:65, :], AF.Ln)
                        rc = rpool.tile([1, SBLK], f32, name="rc")
                        nc.scalar.activation(
                            rc[:], lz[:], AF.Exp, scale=-1.0
                        )
                    else:
                        rc = rpool.tile([1, SBLK], f32, name="rc")
                        nc.vector.reciprocal(rc[:], us[hh][64:65, :])
                    rcs[(qb, g, hh)] = rc

            def emit_norm_rest(qb, g):
                ctx2 = ctx2s[qb]
                us = ctxus.pop((qb, g))
                for hh in range(2):
                    hr = hh * 64
                    rc = rcs.pop((qb, g, hh))
                    pb = ps_s.tile([64, SBLK], f32, tag="s", name="pb")
                    nc.tensor.matmul(
                        pb[:], ones_f32[:], rc[:], start=True, stop=True
                    )
                    rb = rpool.tile([64, SBLK], f32, name="rb")
                    nc.vector.tensor_copy(rb[:], pb[:])
                    nc.vector.tensor_mul(
                        ctx2[hr : hr + 64, g, :], us[hh][0:64, :], rb[:]
                    )
                nc.vector.tensor_scalar_add(
                    ctx2[:, g, :], ctx2[:, g, :], bv_sb[:, g : g + 1]
                )

            def emit_o_proj(qb):
                ctx2 = ctx2s.pop(qb)
                for qs in range(4):
                    ost = opool.tile([P, D], f32, name="ost")
                    for nch in range(2):
                        po = ps_k.tile([P, SBLK], f32, tag="k", name="po")
                        for g in range(NG):
                            nc.tensor.matmul(
                                po[:],
                                ctx2[:, g, ts(qs, P)],
                                wo_sb[:, g, ts(nch, SBLK)],
                                start=(g == 0),
                                stop=(g == NG - 1),
                            )
                        nc.vector.tensor_copy(ost[:, ts(nch, SBLK)], po[:])
                    nc.sync.dma_start(
                        y[ds(qb * SBLK + qs * P, P), :], ost[:]
                    )

            # K/V blocks and later q-block projections are emitted INSIDE
            # the attention stream: the first unit's scores chase the K/V
            # production block by block, so ACT starts exp-ing ~40us
            # earlier, and the projections act as PE filler between
            # ACT-paced iterations.
            emit_kv(0)
            emit_qproj(0)
            steps = [
                (qb, g, kt)
                for qb in range(NSB)
                for g in range(NG)
                for kt in range(NKT)
            ]
            for i, (qb, g, kt) in enumerate(steps):
                emit_scores_exp(qb, g, kt)
                if i == 2:
                    emit_kv(1)
                elif i == 4:
                    nc.sync.dma_start(
                        wo_sb[:], wo.rearrange("(g p) n -> p g n", p=P)
                    )
                elif i == 6:
                    emit_kv(2)
                elif i == 10:
                    emit_kv(3)
                elif i == 18:
                    emit_qproj(1)
                elif i == 34:
                    emit_qproj(2)
                elif i == 66:
                    emit_qproj(3)
                if i >= LAG:
                    pqb, pg, pkt = steps[i - LAG]
                    emit_ctx(pqb, pg, pkt)
                    if pkt == NKT - 1:
                        emit_evac(pqb, pg)
                if kt == 8:
                    # the previous unit's reciprocal has cleared DVE by now
                    if g == 1:
                        emit_norm_rest(qb, 0)
                    elif qb > 0:
                        emit_norm_rest(qb - 1, 1)
                if kt == 12 and g == 0 and qb > 0:
                    emit_o_proj(qb - 1)
            for j in range(len(steps) - LAG, len(steps)):
                qb, g, kt = steps[j]
                emit_ctx(qb, g, kt)
                if kt == NKT - 1:
                    emit_evac(qb, g)

            emit_norm_rest(NSB - 1, 1)
            emit_o_proj(NSB - 1)

    import concourse.mybir as mybir
    from concourse.bass import ds, ts
    from concourse.tile import TileContext

    f32 = mybir.dt.float32
    f32r = mybir.dt.float32r
    AF = mybir.ActivationFunctionType

    nc = bass.Bass()
    xqT = nc.declare_dram_parameter("xqT", [D, S], f32r, isOutput=False)
    xkT = nc.declare_dram_parameter("xkT", [D, S], f32r, isOutput=False)
    xvT = nc.declare_dram_parameter("xvT", [D, S], f32r, isOutput=False)
    wq = nc.declare_dram_parameter("wq", [D, DLOC], f32r, isOutput=False)
    wk = nc.declare_dram_parameter("wk", [D, DLOC], f32r, isOutput=False)
    wv = nc.declare_dram_parameter("wv", [D, DLOC], f32r, isOutput=False)
    wo = nc.declare_dram_parameter("wo", [DLOC, D], f32r, isOutput=False)
    bq = nc.declare_dram_parameter("bq", [DLOC], f32, isOutput=False)
    bk = nc.declare_dram_parameter("bk", [DLOC], f32, isOutput=False)
    bv = nc.declare_dram_parameter("bv", [DLOC], f32, isOutput=False)
    y = nc.declare_dram_parameter("y", [S, D], f32, isOutput=True)

    with TileContext(nc) as tc:
        with (
            tc.tile_pool(name="const", bufs=1) as const,
            tc.tile_pool(name="kv", bufs=1) as kv,
            tc.tile_pool(name="xstr", bufs=3) as xstr,
            tc.tile_pool(name="epool", bufs=5) as epool,
            tc.tile_pool(name="cpool", bufs=2) as cpool,
            tc.tile_pool(name="upool", bufs=2) as upool,
            tc.tile_pool(name="rpool", bufs=2) as rpool,
            tc.tile_pool(name="opool", bufs=2) as opool,
            tc.tile_pool(name="ps_k", bufs=2, space="PSUM") as ps_k,
            tc.tile_pool(name="ps_s", bufs=2, space="PSUM") as ps_s,
            tc.tile_pool(name="ps_c", bufs=2, space="PSUM") as ps_c,
        ):
            # ---- warmup: trip the PE HAM activity monitor to full clock
            # while the first DMAs stream in.
            warm_in = const.tile([P, P], f32)
            nc.gpsimd.memset(warm_in, 0.0)
            warm_ps = ps_k.tile([P, SBLK], f32, tag="k", name="warm")
            for _ in range(48):
                nc.tensor.matmul(
                    warm_ps[:, 0:P], warm_in[:], warm_in[:],
                    start=True, stop=True,
                )

            # ---- constants / weights (DMA'd straight in as f32r) ---------
            ones_f32 = const.tile([1, 64], f32)
            nc.gpsimd.memset(ones_f32, 1.0)
            ones_col = const.tile([1, 64], f32r)
            nc.vector.tensor_copy(ones_col[:], ones_f32[:])
            one_sb = const.tile([P, 1], f32)
            nc.gpsimd.memset(one_sb, 1.0)

            def load_weight(dram, name):
                r = const.tile([P, DCH, DLOC], f32r, name=f"w_{name}")
                nc.sync.dma_start(
                    r[:], dram.rearrange("(dc p) n -> p dc n", p=P)
                )
                return r

            # K/V path loads first: they gate the first scores
            wk_sb = load_weight(wk, "k")
            wv_sb = load_weight(wv, "v")
            bk_sb = const.tile([P, NG], f32)
            nc.sync.dma_start(bk_sb[:], bk.rearrange("(g p) -> p g", p=P))
            bv_sb = const.tile([P, NG], f32)
            nc.sync.dma_start(bv_sb[:], bv.rearrange("(g p) -> p g", p=P))
            wq_sb = load_weight(wq, "q")
            bq_sb = const.tile([P, NG], f32)
            nc.sync.dma_start(bq_sb[:], bq.rearrange("(g p) -> p g", p=P))
            # wo is not needed until the first o_proj (~half way in); its
            # DMA is emitted inside the attention stream
            wo_sb = const.tile([P, NG, D], f32r)

            # persistent Q/K/V state
            qhT = kv.tile([P, NG, S], f32r)
            khT = [
                kv.tile([P, S], f32r, tag=f"khT{g}", name=f"khT{g}")
                for g in range(NG)
            ]
            # vh_aug: [sk-part, kt, head*65] with col 64 of each head == 1.0
            vh_aug = kv.tile([P, NKT, NH * 65], f32r)
            vh4 = vh_aug[:].rearrange("p k (h e) -> p k h e", e=65)
            nc.vector.tensor_copy(
                vh4[:, :, :, 64], one_sb[:].to_broadcast([P, NKT, NH])
            )

            def stream_xT(dram, blk, name):
                t = xstr.tile([P, DCH, SBLK], f32r, tag="xstr", name=name)
                view = dram.rearrange("(dc p) s -> p dc s", p=P)
                for dc in range(DCH):
                    nc.sync.dma_start(
                        t[:, dc], view[:, dc, ds(blk * SBLK, SBLK)]
                    )
                return t

            def emit_qproj(qb):
                xq_blk = stream_xT(xqT, qb, "xq")
                for g in range(NG):
                    pq = ps_k.tile([P, SBLK], f32, tag="k", name="pq")
                    for dc in range(DCH):
                        nc.tensor.matmul(
                            pq[:],
                            wq_sb[:, dc, ts(g, P)],
                            xq_blk[:, dc, :],
                            start=(dc == 0),
                            stop=(dc == DCH - 1),
                        )
                    nc.vector.tensor_scalar_add(
                        qhT[:, g, ts(qb, SBLK)], pq[:], bq_sb[:, g : g + 1]
                    )

            def emit_kv(sb):
                xk_blk = stream_xT(xkT, sb, "xk")
                for g in range(NG):
                    pk = ps_k.tile([P, SBLK], f32, tag="k", name="pk")
                    for dc in range(DCH):
                        nc.tensor.matmul(
                            pk[:],
                            wk_sb[:, dc, ts(g, P)],
                            xk_blk[:, dc, :],
                            start=(dc == 0),
                            stop=(dc == DCH - 1),
                        )
                    nc.vector.tensor_scalar_add(
                        khT[g][:, ts(sb, SBLK)], pk[:], bk_sb[:, g : g + 1]
                    )
                xv_blk = stream_xT(xvT, sb, "xv")
                for ss in range(4):
                    pv = ps_k.tile([P, DLOC], f32, tag="k", name="pv")
                    for dc in range(DCH):
                        nc.tensor.matmul(
                            pv[:],
                            xv_blk[:, dc, ts(ss, P)],
                            wv_sb[:, dc, :],
                            start=(dc == 0),
                            stop=(dc == DCH - 1),
                        )
                    kt = sb * 4 + ss
                    nc.vector.tensor_copy(
                        vh4[:, kt, :, 0:64],
                        pv[:].rearrange("p (h e) -> p h e", e=64),
                    )

            # ---- attention + o_proj: one continuous pipeline -------------
            LAG = 3
            pcs = {}
            exs = {}
            ctx2s = {}
            ctxus = {}
            rcs = {}

            def emit_scores_exp(qb, g, kt):
                ps2 = ps_s.tile([P, 2 * SBLK], f32, tag="s", name="ps2")
                for hh in range(2):
                    hr = hh * 64
                    nc.tensor.matmul(
                        ps2[:, ts(hh, SBLK)],
                        khT[g][hr : hr + 64, ts(kt, P)],
                        qhT[hr : hr + 64, g, ts(qb, SBLK)],
                        start=True,
                        stop=True,
                        tile_position=(hr, 0),
                    )
                ex = epool.tile([P, 2 * SBLK], f32r, name="ex")
                nc.scalar.activation(ex[:], ps2[:], AF.Exp, scale=0.125)
                exs[(qb, g, kt)] = ex

            def emit_ctx(qb, g, kt):
                if kt == 0:
                    pcs[(qb, g)] = [
                        ps_c.tile([P, SBLK], f32, tag="c", name=f"pc{hh}")
                        for hh in range(2)
                    ]
                    if g == 0:
                        ctx2s[qb] = cpool.tile(
                            [P, NG, SBLK], f32r, name="ctx2"
                        )
                ex = exs.pop((qb, g, kt))
                for hh in range(2):
                    h = 2 * g + hh
                    nc.tensor.matmul(
                        pcs[(qb, g)][hh][0:65, :],
                        vh_aug[:, kt, h * 65 : h * 65 + 65],
                        ex[:, ts(hh, SBLK)],
                        start=(kt == 0),
                        stop=(kt == NKT - 1),
                    )

            def emit_evac(qb, g):
                # Evacuate the finished ctx psum pair to SBUF with two
                # cheap DVE copies: frees the accumulation banks without
                # waiting on the normalize. 1/Z runs on the DVE (exact
                # iterative divide, ~3.4us for [1,512]) — its latency hides
                # behind the following unit's stream, and it keeps the ACT
                # queue free for the pipeline-critical exps.
                us = []
                for hh in range(2):
                    pc = pcs[(qb, g)][hh]
                    u = upool.tile([65, SBLK], f32, tag=f"u{hh}", name="u")
                    nc.vector.tensor_copy(u[:], pc[0:65, :])
                    us.append(u)
                del pcs[(qb, g)]
                ctxus[(qb, g)] = us
                for hh in range(2):
                    if (qb, g) == (NSB - 1, NG - 1):
                        # tail unit: ACT is idle by now; exp(-ln Z) avoids
                        # the 3.4us DVE reciprocal on the critical tail
                        lz = rpool.tile([1, SBLK], f32, name="lz")
                        nc.scalar.activation(lz[:], us[hh][64:65, :], AF.Ln)
                        rc = rpool.tile([1, SBLK], f32, name="rc")
                        nc.scalar.activation(
                            rc[:], lz[:], AF.Exp, scale=-1.0
                        )
                    else:
                        rc = rpool.tile([1, SBLK], f32, name="rc")
                        nc.vector.reciprocal(rc[:], us[hh][64:65, :])
                    rcs[(qb, g, hh)] = rc

            def emit_norm_rest(qb, g):
                ctx2 = ctx2s[qb]
                us = ctxus.pop((qb, g))
                for hh in range(2):
                    hr = hh * 64
                    rc = rcs.pop((qb, g, hh))
                    pb = ps_s.tile([64, SBLK], f32, tag="s", name="pb")
                    nc.tensor.matmul(
                        pb[:], ones_f32[:], rc[:], start=True, stop=True
                    )
                    rb = rpool.tile([64, SBLK], f32, name="rb")
                    nc.vector.tensor_copy(rb[:], pb[:])
                    nc.vector.tensor_mul(
                        ctx2[hr : hr + 64, g, :], us[hh][0:64, :], rb[:]
                    )
                nc.vector.tensor_scalar_add(
                    ctx2[:, g, :], ctx2[:, g, :], bv_sb[:, g : g + 1]
                )

            def emit_o_proj(qb):
                ctx2 = ctx2s.pop(qb)
                for qs in range(4):
                    ost = opool.tile([P, D], f32, name="ost")
                    for nch in range(2):
                        po = ps_k.tile([P, SBLK], f32, tag="k", name="po")
                        for g in range(NG):
                            nc.tensor.matmul(
                                po[:],
                                ctx2[:, g, ts(qs, P)],
                                wo_sb[:, g, ts(nch, SBLK)],
                                start=(g == 0),
                                stop=(g == NG - 1),
                            )
                        nc.vector.tensor_copy(ost[:, ts(nch, SBLK)], po[:])
                    nc.sync.dma_start(
                        y[ds(qb * SBLK + qs * P, P), :], ost[:]
                    )

            # K/V blocks and later q-block projections are emitted INSIDE
            # the attention stream: the first unit's scores chase the K/V
            # production block by block, so ACT starts exp-ing ~40us
            # earlier, and the projections act as PE filler between
            # ACT-paced iterations.
            emit_kv(0)
            emit_qproj(0)
            steps = [
                (qb, g, kt)
                for qb in range(NSB)
                for g in range(NG)
                for kt in range(NKT)
            ]
            for i, (qb, g, kt) in enumerate(steps):
                emit_scores_exp(qb, g, kt)
                if i == 2:
                    emit_kv(1)
                elif i == 4:
                    nc.sync.dma_start(
                        wo_sb[:], wo.rearrange("(g p) n -> p g n", p=P)
                    )
                elif i == 6:
                    emit_kv(2)
                elif i == 10:
                    emit_kv(3)
                elif i == 18:
                    emit_qproj(1)
                elif i == 34:
                    emit_qproj(2)
                elif i == 66:
                    emit_qproj(3)
                if i >= LAG:
                    pqb, pg, pkt = steps[i - LAG]
                    emit_ctx(pqb, pg, pkt)
                    if pkt == NKT - 1:
                        emit_evac(pqb, pg)
                if kt == 8:
                    # the previous unit's reciprocal has cleared DVE by now
                    if g == 1:
                        emit_norm_rest(qb, 0)
                    elif qb > 0:
                        emit_norm_rest(qb - 1, 1)
                if kt == 12 and g == 0 and qb > 0:
                    emit_o_proj(qb - 1)
            for j in range(len(steps) - LAG, len(steps)):
                qb, g, kt = steps[j]
                emit_ctx(qb, g, kt)
                if kt == NKT - 1:
                    emit_evac(qb, g)

            # fused tail for the last unit: normalize and o_proj pipeline
            # per 128-column chunk instead of running the whole normalize
            # before the first o_proj matmul
            qb, g = NSB - 1, 1
            ctx2 = ctx2s.pop(qb)
            us = ctxus.pop((qb, g))
            rbs = []
            for hh in range(2):
                rc = rcs.pop((qb, g, hh))
                pb = ps_s.tile([64, SBLK], f32, tag="s", name="pb")
                nc.tensor.matmul(
                    pb[:], ones_f32[:], rc[:], start=True, stop=True
                )
                rb = rpool.tile([64, SBLK], f32, name="rb")
                nc.vector.tensor_copy(rb[:], pb[:])
                rbs.append(rb)
            for qs in range(4):
                qsl = ts(qs, P)
                for hh in range(2):
                    hr = hh * 64
                    nc.vector.tensor_mul(
                        ctx2[hr : hr + 64, g, qsl],
                        us[hh][0:64, qsl],
                        rbs[hh][:, qsl],
                    )
                nc.vector.tensor_scalar_add(
                    ctx2[:, g, qsl], ctx2[:, g, qsl], bv_sb[:, g : g + 1]
                )
                ost = opool.tile([P, D], f32, name="ost")
                for nch in range(2):
                    po = ps_k.tile([P, SBLK], f32, tag="k", name="po")
                    for gg in range(NG):
                        nc.tensor.matmul(
                            po[:],
                            ctx2[:, gg, qsl],
                            wo_sb[:, gg, ts(nch, SBLK)],
                            start=(gg == 0),
                            stop=(gg == NG - 1),
                        )
                    nc.vector.tensor_copy(ost[:, ts(nch, SBLK)], po[:])
                nc.sync.dma_start(y[ds(qb * SBLK + qs * P, P), :], ost[:])

    import concourse.mybir as mybir

    _split_excess_waits(nc, mybir)
    return nc


def kernel(q, k, v, Wq, bq, Wk, bk, Wv, bv, Wo, bo):
    from concourse.bass_utils import run_bass_kernel_spmd

    q = np.asarray(q, dtype=np.float32)
    k = np.asarray(k, dtype=np.float32)
    v = np.asarray(v, dtype=np.float32)
    Wq = np.asarray(Wq, dtype=np.float32)
    Wk = np.asarray(Wk, dtype=np.float32)
    Wv = np.asarray(Wv, dtype=np.float32)
    Wo = np.asarray(Wo, dtype=np.float32)
    bq = np.asarray(bq, dtype=np.float32)
    bk = np.asarray(bk, dtype=np.float32)
    bv = np.asarray(bv, dtype=np.float32)
    bo = np.asarray(bo, dtype=np.float32)

    if "nc" not in _program_cache:
        _program_cache["nc"] = _build_program()
    nc = _program_cache["nc"]

    qT = [np.ascontiguousarray(q[b].T) for b in range(B)]
    kT = [np.ascontiguousarray(k[b].T) for b in range(B)]
    vT = [np.ascontiguousarray(v[b].T) for b in range(B)]

    in_maps = []
    for c in range(8):
        b, hg = c // 4, c % 4
        cols = slice(DLOC * hg, DLOC * (hg + 1))
        in_maps.append(
            {
                "xqT": qT[b],
                "xkT": kT[b],
                "xvT": vT[b],
                "wq": np.ascontiguousarray(Wq[:, cols]),
                "wk": np.ascontiguousarray(Wk[:, cols]),
                "wv": np.ascontiguousarray(Wv[:, cols]),
                "wo": np.ascontiguousarray(Wo[cols, :]),
                "bq": np.ascontiguousarray(bq[cols]),
                "bk": np.ascontiguousarray(bk[cols]),
                "bv": np.ascontiguousarray(bv[cols]),
            }
        )

    global _last_in_maps
    _last_in_maps = in_maps

    res = run_bass_kernel_spmd(nc, in_maps, list(range(8)))

    out = np.empty((B, S, D), np.float32)
    for b in range(B):
        acc = res.results[4 * b]["y"].astype(np.float32).copy()
        for hg in range(1, 4):
            acc += res.results[4 * b + hg]["y"]
        out[b] = acc + bo[None, :]
    return out
